# revision 15
# baseline (speedup 1.0000x reference)
"""Trainium2 Bass kernel for nn_Attention_55740085567875.

Full attention block: qkv = x @ W_qkv.T + b; q,k head-layernormed (qk_norm bug
in the source applies LN to q and k); attn = softmax(q k^T / sqrt(D)); returns
(out = attn @ v @ W_proj.T + b_proj, attn_weights).

Sharding: 8 cores = (batch b in 0..4) x (query-token half r in 0..2).
Each core computes q for its 512 query tokens, k/v for all 1024 tokens of its
batch (k/v compute duplicated across the token-half pair - avoids any
cross-core communication), the 16 heads' attention, softmax weights for its
[16, 512, 1024] slice, and the final projection rows. Host does input
transposes and output concatenation only.

On-chip layout strategy per core:
  - qkv computed in token-major layout [n, c] (stationary = x^T tiles) so the
    per-head layernorm reduces along the free dim.
  - q,k transposed on TensorE to head-major [d, n] for the attention matmuls.
  - attention computed BOTH ways: attnT = k q^T [m, n] feeds exp -> PV
    (with a ones-column appended to V so the softmax denominators fall out of
    the PV matmul for free); attn = q k^T [n, m] feeds one fused
    ACT pass w = Exp(attn - logsum[n]) -> the normalized weights output.
  - all matmuls run in float32r (full PE rate, ~2e-4 component error).
"""

import numpy as np

import concourse.bass as bass
import concourse.mybir as mybir
import concourse.tile as tile
from concourse import bacc
from concourse.bass_utils import run_bass_kernel_spmd
from concourse.masks import make_identity
from contextlib import ExitStack

F32 = mybir.dt.float32
F32R = mybir.dt.float32r
AF = mybir.ActivationFunctionType
OP = mybir.AluOpType

B, N, C, H = 4, 1024, 1024, 16
D = C // H            # 64
NQ = N // 2           # 512 query tokens per core
NT = N // 128         # 8 key-token tiles
NQT = NQ // 128       # 4 query-token tiles
CT = C // 128         # 8 channel tiles
EPS = 1e-5
LN_SCALE = float(np.log(D ** -0.5))  # ln(0.125), folded into q's rstd

N_CORES = 8


def _build(has_bqkv, has_bproj, has_qgb, has_kgb):
    nc = bacc.Bacc("TRN2", target_bir_lowering=False, debug=False)

    xT = nc.dram_tensor("xT", [C, N], F32R, kind="ExternalInput").ap()
    xTq = nc.dram_tensor("xTq", [C, NQ], F32R, kind="ExternalInput").ap()
    wqkvT = nc.dram_tensor("wqkvT", [C, 3 * C], F32R, kind="ExternalInput").ap()
    wprojT = nc.dram_tensor("wprojT", [C, C], F32R, kind="ExternalInput").ap()
    ind = nc.dram_tensor("ind", [H, CT, 128], F32R, kind="ExternalInput").ap()
    idn = nc.dram_tensor("idn", [128, 128], F32R, kind="ExternalInput").ap()
    onesr = nc.dram_tensor("onesr", [1, 128], F32R, kind="ExternalInput").ap()
    vones = nc.dram_tensor("vones", [128, H], F32R, kind="ExternalInput").ap()
    bqkv = bproj = qg = qb = kg = kb = None
    if has_bqkv:
        bqkv = nc.dram_tensor("bqkv", [1, 3 * C], F32R, kind="ExternalInput").ap()
    if has_bproj:
        bproj = nc.dram_tensor("bproj", [1, C], F32R, kind="ExternalInput").ap()
    if has_qgb:
        qg = nc.dram_tensor("qg", [128, C], F32, kind="ExternalInput").ap()
        qb = nc.dram_tensor("qb", [128, C], F32, kind="ExternalInput").ap()
    if has_kgb:
        kg = nc.dram_tensor("kg", [128, C], F32, kind="ExternalInput").ap()
        kb = nc.dram_tensor("kb", [128, C], F32, kind="ExternalInput").ap()

    wout = nc.dram_tensor("w_part", [H, NQ, N], F32, kind="ExternalOutput").ap()
    oout = nc.dram_tensor("out_part", [NQ, C], F32, kind="ExternalOutput").ap()

    with tile.TileContext(nc) as tc, ExitStack() as top:
        top.enter_context(nc.allow_low_precision(
            reason="float32r rounding of matmul operands is intentional"))
        pp = top.enter_context(tc.tile_pool(name="pp", bufs=1))

        # constants
        identR = pp.tile([128, 128], F32R, name="identR", tag="identR")
        nc.sync.dma_start(out=identR, in_=idn)
        ones_row = pp.tile([1, 128], F32R, name="ones_row", tag="ones_row")
        nc.sync.dma_start(out=ones_row, in_=onesr)
        ind_sb = pp.tile([H, CT, 128], F32R, name="ind_sb", tag="ind_sb")
        eps_t = pp.tile([128, 1], F32, name="eps_t", tag="eps_t")
        nc.vector.memset(eps_t, EPS)
        lnsc_t = pp.tile([128, 1], F32, name="lnsc_t", tag="lnsc_t")
        nc.vector.memset(lnsc_t, LN_SCALE)
        nc.sync.dma_start(out=ind_sb, in_=ind)
        if has_bqkv:
            bqkv_sb = pp.tile([1, 3 * C], F32R, name="bqkv_sb", tag="bqkv_sb")
            nc.sync.dma_start(out=bqkv_sb, in_=bqkv)
        if has_bproj:
            bproj_sb = pp.tile([1, C], F32R, name="bproj_sb", tag="bproj_sb")
            nc.sync.dma_start(out=bproj_sb, in_=bproj)
        if has_qgb:
            qg_sb = pp.tile([128, C], F32, name="qg_sb", tag="qg_sb")
            qb_sb = pp.tile([128, C], F32, name="qb_sb", tag="qb_sb")
            nc.sync.dma_start(out=qg_sb, in_=qg)
            nc.sync.dma_start(out=qb_sb, in_=qb)
        if has_kgb:
            kg_sb = pp.tile([128, C], F32, name="kg_sb", tag="kg_sb")
            kb_sb = pp.tile([128, C], F32, name="kb_sb", tag="kb_sb")
            nc.sync.dma_start(out=kg_sb, in_=kg)
            nc.sync.dma_start(out=kb_sb, in_=kb)

        # persistent per-core arrays
        vput = [pp.tile([128, H, D + 1], F32R, name=f"vaug{i}", tag=f"vaug{i}") for i in range(NT)]
        for t in range(NT):
            nc.sync.dma_start(out=vput[t][:, :, D:D + 1],
                              in_=vones[:, :].unsqueeze(2))

        # ---------------- S1: qkv in token-major layout ----------------
        pqk_stack = top.enter_context(ExitStack())
        pqk = pqk_stack.enter_context(tc.tile_pool(name="pqk", bufs=1))
        s1_stack = top.enter_context(ExitStack())
        px = s1_stack.enter_context(tc.tile_pool(name="px", bufs=1))
        pw1 = s1_stack.enter_context(tc.tile_pool(name="pw1", bufs=2))
        ps1 = s1_stack.enter_context(tc.tile_pool(name="ps1", bufs=4, space="PSUM"))
        if True:
            xT_t = [px.tile([128, N], F32R, name=f"xT{i}", tag=f"xT{i}") for i in range(CT)]
            xTq_t = [px.tile([128, NQ], F32R, name=f"xTq{i}", tag=f"xTq{i}") for i in range(CT)]
            for i in range(CT):
                nc.sync.dma_start(out=xT_t[i], in_=xT[i * 128:(i + 1) * 128, :])
                nc.sync.dma_start(out=xTq_t[i], in_=xTq[i * 128:(i + 1) * 128, :])

            q_sb = [pqk.tile([128, C], F32R, name=f"q{i}", tag=f"q{i}") for i in range(NQT)]
            k_sb = [pqk.tile([128, C], F32R, name=f"k{i}", tag=f"k{i}") for i in range(NT)]

            for ch in range(6):  # q0 q1 k0 k1 v0 v1
                is_q = ch < 2
                is_k = 2 <= ch < 4
                wt = [pw1.tile([128, 512], F32R, name=f"wt{ft}", tag=f"wt{ft}") for ft in range(CT)]
                for ft in range(CT):
                    nc.sync.dma_start(
                        out=wt[ft],
                        in_=wqkvT[ft * 128:(ft + 1) * 128, ch * 512:(ch + 1) * 512])
                ntiles = NQT if is_q else NT
                for nt in range(ntiles):
                    ps = ps1.tile([128, 512], F32)
                    for ft in range(CT):
                        lhsT = (xTq_t[ft] if is_q else xT_t[ft])[:, nt * 128:(nt + 1) * 128]
                        nc.tensor.matmul(ps[:, :], lhsT, wt[ft][:, :],
                                         start=(ft == 0),
                                         stop=(ft == CT - 1 and not has_bqkv))
                    if has_bqkv:
                        nc.tensor.matmul(
                            ps[:, :], ones_row[:, :],
                            bqkv_sb[:, ch * 512:(ch + 1) * 512],
                            start=False, stop=True)
                    half = ch % 2
                    if is_q:
                        nc.scalar.copy(q_sb[nt][:, half * 512:(half + 1) * 512], ps[:, :])
                    elif is_k:
                        nc.scalar.copy(k_sb[nt][:, half * 512:(half + 1) * 512], ps[:, :])
                    else:
                        nc.vector.tensor_copy(
                            vput[nt][:, half * 8:half * 8 + 8, 0:D],
                            ps[:, :].rearrange("p (h d) -> p h d", h=8))

            # ---------------- S2: layernorm on q, k (free-dim per head) ----
            s1_stack.close()
            with tc.tile_pool(name="pln", bufs=3) as pln:
                for tiles, ntl, is_q_ln in ((q_sb, NQT, True), (k_sb, NT, False)):
                    for nt in range(ntl):
                        src = tiles[nt]
                        src3 = src[:, :].rearrange("p (h d) -> p h d", h=H)
                        sums = pln.tile([128, H], F32, name="sums", tag="sums")
                        nc.vector.reduce_sum(sums[:, :], src3, axis=mybir.AxisListType.X)
                        sq = pln.tile([128, C], F32, name="sq", tag="sq")
                        nc.scalar.square(sq[:, :], src[:, :])
                        sumsq = pln.tile([128, H], F32, name="sumsq", tag="sumsq")
                        nc.vector.reduce_sum(
                            sumsq[:, :], sq[:, :].rearrange("p (h d) -> p h d", h=H),
                            axis=mybir.AxisListType.X)
                        mean = pln.tile([128, H], F32, name="mean", tag="mean")
                        nc.vector.tensor_scalar_mul(mean[:, :], sums[:, :], 1.0 / D)
                        # var = sumsq/D - mean^2  (population variance)
                        msq = pln.tile([128, H], F32, name="msq", tag="msq")
                        nc.vector.tensor_mul(msq[:, :], mean[:, :], mean[:, :])
                        var = pln.tile([128, H], F32, name="var", tag="var")
                        nc.vector.scalar_tensor_tensor(
                            out=var[:, :], in0=sumsq[:, :], scalar=1.0 / D,
                            in1=msq[:, :], op0=OP.mult, op1=OP.subtract)
                        # rstd' = exp(-0.5*ln(var+eps) [+ ln(scale) for q])
                        lnv = pln.tile([128, H], F32, name="lnv", tag="lnv")
                        nc.scalar.activation(lnv[:, :], var[:, :], AF.Ln, bias=eps_t[:, :])
                        rstd = pln.tile([128, H], F32, name="rstd", tag="rstd")
                        nc.scalar.activation(rstd[:, :], lnv[:, :], AF.Exp,
                                             bias=(lnsc_t[:, :] if is_q_ln else 0.0),
                                             scale=-0.5)
                        dstR = src[:, :]
                        for h in range(H):
                            nc.vector.tensor_scalar(
                                out=dstR[:, h * D:(h + 1) * D],
                                in0=src[:, h * D:(h + 1) * D],
                                scalar1=mean[:, h:h + 1],
                                scalar2=rstd[:, h:h + 1],
                                op0=OP.subtract, op1=OP.mult)
                        if is_q_ln and has_qgb:
                            nc.vector.tensor_mul(dstR, dstR, qg_sb[:, :])
                            nc.vector.tensor_add(dstR, dstR, qb_sb[:, :])
                        if (not is_q_ln) and has_kgb:
                            nc.vector.tensor_mul(dstR, dstR, kg_sb[:, :])
                            nc.vector.tensor_add(dstR, dstR, kb_sb[:, :])

            # ---------------- S3: transpose q,k to head-major [d, n] -------
            patt = top.enter_context(tc.tile_pool(name="patt", bufs=1))
            qT = [patt.tile([128, NQ], F32R, name=f"qT{i}", tag=f"qT{i}") for i in range(CT)]
            kT = [patt.tile([128, N], F32R, name=f"kT{i}", tag=f"kT{i}") for i in range(CT)]
            with tc.tile_pool(name="ps3", bufs=2, space="PSUM") as ps3, \
                 tc.tile_pool(name="ps3k", bufs=2, space="PSUM") as ps3k:
                for ct in range(CT):
                    psq = ps3.tile([128, NQ], F32R)
                    for nt in range(NQT):
                        nc.tensor.transpose(
                            psq[:, nt * 128:(nt + 1) * 128],
                            q_sb[nt][:, ct * 128:(ct + 1) * 128],
                            identR[:, :])
                    nc.vector.tensor_copy(qT[ct][:, :], psq[:, :])
                    psk = ps3k.tile([128, N], F32R)
                    for nt in range(NT):
                        nc.tensor.transpose(
                            psk[:, nt * 128:(nt + 1) * 128],
                            k_sb[nt][:, ct * 128:(ct + 1) * 128],
                            identR[:, :])
                    nc.vector.tensor_copy(kT[ct][:, :], psk[:, :])

        # ---------------- S4: attnT = k q^T per head; exp; PV --------------
        # q_sb/k_sb raw values are dead after S3 - reuse their storage for the
        # (un)normalized attention outputs.
        aoutU = [q_sb[i // 2][:, (i % 2) * NQ:(i % 2) * NQ + NQ] for i in range(CT)]
        aoutN = [k_sb[i // 2][:, (i % 2) * NQ:(i % 2) * NQ + NQ] for i in range(CT)]
        sumsT = patt.tile([H, NQ], F32, name="sumsT", tag="sumsT")

        def head_slices(h):
            t, o = divmod(h, 2)
            return t, o * 64

        with tc.tile_pool(name="pexp", bufs=6) as pexp, \
             tc.tile_pool(name="pskq", bufs=2, space="PSUM") as pskq, \
             tc.tile_pool(name="pspv", bufs=2, space="PSUM") as pspv:
            for h in range(H):
                t, o = head_slices(h)
                expT = []
                for j in range(NT // 2):
                    pkq = pskq.tile([128, 1024], F32)
                    for s in range(2):
                        mt = 2 * j + s
                        nc.tensor.matmul(
                            pkq[:, s * 512:(s + 1) * 512],
                            kT[t][o:o + 64, mt * 128:(mt + 1) * 128],
                            qT[t][o:o + 64, :],
                            start=True, stop=True)
                    et = pexp.tile([128, 1024], F32R, name="expT", tag="expT")
                    nc.scalar.activation(et[:, :], pkq[:, :], AF.Exp)
                    expT.append(et)
                ppv = pspv.tile([D + 1, NQ], F32)
                for mt in range(NT):
                    nc.tensor.matmul(
                        ppv[:, :],
                        vput[mt][:, h, :],
                        expT[mt // 2][:, (mt % 2) * 512:(mt % 2) * 512 + 512],
                        start=(mt == 0), stop=(mt == NT - 1))
                stmp = pexp.tile([1, NQ], F32, name="stmp", tag="stmp", bufs=3)
                nc.vector.tensor_copy(stmp[:, :], ppv[D:D + 1, :])
                nc.sync.dma_start(out=sumsT[h:h + 1, :], in_=stmp[:, :])
                nc.vector.tensor_copy(aoutU[t][o:o + 64, :], ppv[0:D, :])

        # softmax denominators -> recip (for PV normalize) + -logsum (for w)
        recipT = patt.tile([H, NQ], F32R, name="recipT", tag="recipT")
        nc.vector.reciprocal(recipT[:, :], sumsT[:, :])
        neglog = patt.tile([H, NQ], F32, name="neglog", tag="neglog")
        nc.scalar.activation(neglog[:, :], recipT[:, :], AF.Ln)
        nlT = [patt.tile([128, H], F32, name=f"nlT{i}", tag=f"nlT{i}") for i in range(NQT)]
        with tc.tile_pool(name="psnl", bufs=2, space="PSUM") as psnl, \
             tc.tile_pool(name="psbc", bufs=2, space="PSUM") as psbc:
            for nt in range(NQT):
                pnl = psnl.tile([128, H], F32)
                nc.tensor.transpose(
                    pnl[:, :], neglog[:, nt * 128:(nt + 1) * 128],
                    identR[:, :].bitcast(F32)[0:H, 0:H])
                nc.vector.tensor_copy(nlT[nt][:, :], pnl[:, :])
            for ct in range(CT):
                pbc = psbc.tile([128, NQ], F32)
                nc.tensor.matmul(pbc[:, :], ind_sb[:, ct, :], recipT[:, :],
                                 start=True, stop=True)
                nc.vector.tensor_mul(aoutN[ct], aoutU[ct], pbc[:, :])

        # ---------------- S5: attn = q k^T; w = Exp(attn - logsum); store --
        # ---------------- S6: out = attnout @ W_proj.T + b_proj ------------
        with tc.tile_pool(name="pw5", bufs=4) as pw5, \
             tc.tile_pool(name="pwp", bufs=1) as pwp, \
             tc.tile_pool(name="pob", bufs=2) as pob, \
             tc.tile_pool(name="psat", bufs=2, space="PSUM") as psat, \
             tc.tile_pool(name="pspj", bufs=2, space="PSUM") as pspj:
            for h in range(H):
                t, o = head_slices(h)
                for nt in range(NQT):
                    pat = psat.tile([128, 1024], F32)
                    for s in range(2):
                        nc.tensor.matmul(
                            pat[:, s * 512:(s + 1) * 512],
                            qT[t][o:o + 64, nt * 128:(nt + 1) * 128],
                            kT[t][o:o + 64, s * 512:(s + 1) * 512],
                            start=True, stop=True)
                    wsb = pw5.tile([128, 1024], F32, name="wsb", tag="wsb")
                    nc.scalar.activation(wsb[:, :], pat[:, :], AF.Exp,
                                         bias=nlT[nt][:, h:h + 1])
                    nc.sync.dma_start(
                        out=wout[h, nt * 128:(nt + 1) * 128, :], in_=wsb[:, :])

            wp = [pwp.tile([128, C], F32R, name=f"wp{i}", tag=f"wp{i}") for i in range(CT)]
            for ct in range(CT):
                nc.sync.dma_start(out=wp[ct], in_=wprojT[ct * 128:(ct + 1) * 128, :])
            for nt in range(NQT):
                ob = pob.tile([128, C], F32, name="ob", tag="ob")
                for chalf in range(2):
                    pj = pspj.tile([128, 512], F32)
                    for ct in range(CT):
                        nc.tensor.matmul(
                            pj[:, :],
                            aoutN[ct][:, nt * 128:(nt + 1) * 128],
                            wp[ct][:, chalf * 512:(chalf + 1) * 512],
                            start=(ct == 0),
                            stop=(ct == CT - 1 and not has_bproj))
                    if has_bproj:
                        nc.tensor.matmul(pj[:, :], ones_row[:, :],
                                         bproj_sb[:, chalf * 512:(chalf + 1) * 512],
                                         start=False, stop=True)
                    nc.vector.tensor_copy(ob[:, chalf * 512:(chalf + 1) * 512], pj[:, :])
                nc.sync.dma_start(out=oout[nt * 128:(nt + 1) * 128, :], in_=ob[:, :])

    nc.compile()
    return nc


_NC_CACHE = {}


def _get_nc(flags):
    if flags not in _NC_CACHE:
        _NC_CACHE[flags] = _build(*flags)
    return _NC_CACHE[flags]


def _flags_of(inputs):
    return (bool(np.any(inputs["b_qkv"] != 0.0)),
            bool(np.any(inputs["b_proj"] != 0.0)),
            bool(np.any(inputs["qn_g"] != 1.0) or np.any(inputs["qn_b"] != 0.0)),
            bool(np.any(inputs["kn_g"] != 1.0) or np.any(inputs["kn_b"] != 0.0)))


def make_in_maps(inputs):
    inputs = {k: np.asarray(v, dtype=np.float32) for k, v in inputs.items()}
    x, W_qkv, b_qkv = inputs["x"], inputs["W_qkv"], inputs["b_qkv"]
    qn_g, qn_b = inputs["qn_g"], inputs["qn_b"]
    kn_g, kn_b = inputs["kn_g"], inputs["kn_b"]
    W_proj, b_proj = inputs["W_proj"], inputs["b_proj"]
    has_bqkv, has_bproj, has_qgb, has_kgb = _flags_of(inputs)

    wqkvT = np.ascontiguousarray(W_qkv.T)              # [C, 3C]
    wprojT = np.ascontiguousarray(W_proj.T)            # [C, C]
    ind = np.zeros((H, CT, 128), np.float32)
    for t in range(CT):
        ind[2 * t, t, 0:64] = 1.0
        ind[2 * t + 1, t, 64:128] = 1.0

    common = {"wqkvT": wqkvT, "wprojT": wprojT, "ind": ind,
              "idn": np.eye(128, dtype=np.float32),
              "onesr": np.ones((1, 128), np.float32),
              "vones": np.ones((128, H), np.float32)}
    if has_bqkv:
        common["bqkv"] = b_qkv.reshape(1, 3 * C)
    if has_bproj:
        common["bproj"] = b_proj.reshape(1, C)
    if has_qgb:
        scale = D ** -0.5
        common["qg"] = np.broadcast_to(np.tile(qn_g, H), (128, C)).copy()
        common["qb"] = np.broadcast_to(np.tile(qn_b * scale, H), (128, C)).copy()
    if has_kgb:
        common["kg"] = np.broadcast_to(np.tile(kn_g, H), (128, C)).copy()
        common["kb"] = np.broadcast_to(np.tile(kn_b, H), (128, C)).copy()

    in_maps = []
    for core in range(N_CORES):
        b, r = divmod(core, 2)
        xTb = np.ascontiguousarray(x[b].T)             # [C, N]
        xTqb = np.ascontiguousarray(xTb[:, r * NQ:(r + 1) * NQ])
        m = dict(common)
        m["xT"] = xTb
        m["xTq"] = xTqb
        in_maps.append(m)
    return in_maps


def kernel(x, W_qkv, b_qkv, qn_g, qn_b, kn_g, kn_b, W_proj, b_proj):
    inputs = dict(x=x, W_qkv=W_qkv, b_qkv=b_qkv, qn_g=qn_g, qn_b=qn_b,
                  kn_g=kn_g, kn_b=kn_b, W_proj=W_proj, b_proj=b_proj)
    inputs = {k: np.asarray(v, dtype=np.float32) for k, v in inputs.items()}
    nc = _get_nc(_flags_of(inputs))
    in_maps = make_in_maps(inputs)

    res = run_bass_kernel_spmd(nc, in_maps, core_ids=list(range(N_CORES)))

    out = np.empty((B, N, C), np.float32)
    weights = np.empty((B, H, N, N), np.float32)
    for core in range(N_CORES):
        b, r = divmod(core, 2)
        out[b, r * NQ:(r + 1) * NQ, :] = res.results[core]["out_part"]
        weights[b, :, r * NQ:(r + 1) * NQ, :] = res.results[core]["w_part"]
    return out, weights


# revision 16
# speedup vs baseline: 1.1468x; 1.1468x over previous
"""Trainium2 Bass kernel for nn_Attention_55740085567875.

Full attention block: qkv = x @ W_qkv.T + b; q,k head-layernormed (qk_norm bug
in the source applies LN to q and k); attn = softmax(q k^T / sqrt(D)); returns
(out = attn @ v @ W_proj.T + b_proj, attn_weights).

Sharding: 8 cores = (batch b in 0..4) x (query-token half r in 0..2).
Each core computes q for its 512 query tokens, k/v for all 1024 tokens of its
batch (k/v compute duplicated across the token-half pair - avoids any
cross-core communication), the 16 heads' attention, softmax weights for its
[16, 512, 1024] slice, and the final projection rows. Host does input
transposes and output concatenation only.

On-chip layout strategy per core:
  - qkv computed in token-major layout [n, c] (stationary = x^T tiles) so the
    per-head layernorm reduces along the free dim.
  - q,k transposed on TensorE to head-major [d, n] for the attention matmuls.
  - attention computed BOTH ways: attnT = k q^T [m, n] feeds exp -> PV
    (with a ones-column appended to V so the softmax denominators fall out of
    the PV matmul for free); attn = q k^T [n, m] feeds one fused
    ACT pass w = Exp(attn - logsum[n]) -> the normalized weights output.
  - all matmuls run in float32r (full PE rate, ~2e-4 component error).
"""

import numpy as np

import concourse.bass as bass
import concourse.mybir as mybir
import concourse.tile as tile
from concourse import bacc
from concourse.bass_utils import run_bass_kernel_spmd
from concourse.masks import make_identity
from contextlib import ExitStack

F32 = mybir.dt.float32
F32R = mybir.dt.float32r
AF = mybir.ActivationFunctionType
OP = mybir.AluOpType

B, N, C, H = 4, 1024, 1024, 16
D = C // H            # 64
NQ = N // 2           # 512 query tokens per core
NT = N // 128         # 8 key-token tiles
NQT = NQ // 128       # 4 query-token tiles
CT = C // 128         # 8 channel tiles
EPS = 1e-5
LN_SCALE = float(np.log(D ** -0.5))  # ln(0.125), folded into q's rstd

N_CORES = 8


def _build(has_bqkv, has_bproj, has_qgb, has_kgb):
    nc = bacc.Bacc("TRN2", target_bir_lowering=False, debug=False)

    xT = nc.dram_tensor("xT", [C, N], F32R, kind="ExternalInput").ap()
    xTq = nc.dram_tensor("xTq", [C, NQ], F32R, kind="ExternalInput").ap()
    wqkvT = nc.dram_tensor("wqkvT", [C, 3 * C], F32R, kind="ExternalInput").ap()
    wprojT = nc.dram_tensor("wprojT", [C, C], F32R, kind="ExternalInput").ap()
    ind = nc.dram_tensor("ind", [H, CT, 128], F32R, kind="ExternalInput").ap()
    idn = nc.dram_tensor("idn", [128, 128], F32R, kind="ExternalInput").ap()
    onesr = nc.dram_tensor("onesr", [1, 128], F32R, kind="ExternalInput").ap()
    vones = nc.dram_tensor("vones", [128, H], F32R, kind="ExternalInput").ap()
    bqkv = bproj = qg = qb = kg = kb = None
    if has_bqkv:
        bqkv = nc.dram_tensor("bqkv", [1, 3 * C], F32R, kind="ExternalInput").ap()
    if has_bproj:
        bproj = nc.dram_tensor("bproj", [1, C], F32R, kind="ExternalInput").ap()
    if has_qgb:
        qg = nc.dram_tensor("qg", [128, C], F32, kind="ExternalInput").ap()
        qb = nc.dram_tensor("qb", [128, C], F32, kind="ExternalInput").ap()
    if has_kgb:
        kg = nc.dram_tensor("kg", [128, C], F32, kind="ExternalInput").ap()
        kb = nc.dram_tensor("kb", [128, C], F32, kind="ExternalInput").ap()

    wout = nc.dram_tensor("w_part", [H, NQ, N], F32, kind="ExternalOutput").ap()
    oout = nc.dram_tensor("out_part", [NQ, C], F32, kind="ExternalOutput").ap()

    with tile.TileContext(nc) as tc, ExitStack() as top:
        top.enter_context(nc.allow_low_precision(
            reason="float32r rounding of matmul operands is intentional"))
        pp = top.enter_context(tc.tile_pool(name="pp", bufs=1))

        # constants
        identR = pp.tile([128, 128], F32R, name="identR", tag="identR")
        nc.sync.dma_start(out=identR, in_=idn)
        ones_row = pp.tile([1, 128], F32R, name="ones_row", tag="ones_row")
        nc.sync.dma_start(out=ones_row, in_=onesr)
        ind_sb = pp.tile([H, CT, 128], F32R, name="ind_sb", tag="ind_sb")
        eps_t = pp.tile([128, 1], F32, name="eps_t", tag="eps_t")
        nc.vector.memset(eps_t, EPS)
        lnsc_t = pp.tile([128, 1], F32, name="lnsc_t", tag="lnsc_t")
        nc.vector.memset(lnsc_t, LN_SCALE)
        nc.sync.dma_start(out=ind_sb, in_=ind)
        if has_bqkv:
            bqkv_sb = pp.tile([1, 3 * C], F32R, name="bqkv_sb", tag="bqkv_sb")
            nc.sync.dma_start(out=bqkv_sb, in_=bqkv)
        if has_bproj:
            bproj_sb = pp.tile([1, C], F32R, name="bproj_sb", tag="bproj_sb")
            nc.sync.dma_start(out=bproj_sb, in_=bproj)
        if has_qgb:
            qg_sb = pp.tile([128, C], F32, name="qg_sb", tag="qg_sb")
            qb_sb = pp.tile([128, C], F32, name="qb_sb", tag="qb_sb")
            nc.sync.dma_start(out=qg_sb, in_=qg)
            nc.sync.dma_start(out=qb_sb, in_=qb)
        if has_kgb:
            kg_sb = pp.tile([128, C], F32, name="kg_sb", tag="kg_sb")
            kb_sb = pp.tile([128, C], F32, name="kb_sb", tag="kb_sb")
            nc.sync.dma_start(out=kg_sb, in_=kg)
            nc.sync.dma_start(out=kb_sb, in_=kb)

        # persistent per-core arrays
        vput = [pp.tile([128, H, D + 1], F32R, name=f"vaug{i}", tag=f"vaug{i}") for i in range(NT)]
        for t in range(NT):
            nc.sync.dma_start(out=vput[t][:, :, D:D + 1],
                              in_=vones[:, :].unsqueeze(2))

        # ---------------- S1: qkv in token-major layout ----------------
        pqk_stack = top.enter_context(ExitStack())
        pqk = pqk_stack.enter_context(tc.tile_pool(name="pqk", bufs=1))
        s1_stack = top.enter_context(ExitStack())
        px = s1_stack.enter_context(tc.tile_pool(name="px", bufs=1))
        pw1 = s1_stack.enter_context(tc.tile_pool(name="pw1", bufs=2))
        ps1 = s1_stack.enter_context(tc.tile_pool(name="ps1", bufs=4, space="PSUM"))
        if True:
            xT_t = [px.tile([128, N], F32R, name=f"xT{i}", tag=f"xT{i}") for i in range(CT)]
            xTq_t = [px.tile([128, NQ], F32R, name=f"xTq{i}", tag=f"xTq{i}") for i in range(CT)]
            for i in range(CT):
                nc.sync.dma_start(out=xT_t[i], in_=xT[i * 128:(i + 1) * 128, :])
                nc.sync.dma_start(out=xTq_t[i], in_=xTq[i * 128:(i + 1) * 128, :])

            q_sb = [pqk.tile([128, C], F32R, name=f"q{i}", tag=f"q{i}") for i in range(NQT)]
            k_sb = [pqk.tile([128, C], F32R, name=f"k{i}", tag=f"k{i}") for i in range(NT)]

            for ch in range(6):  # q0 q1 k0 k1 v0 v1
                is_q = ch < 2
                is_k = 2 <= ch < 4
                wt = [pw1.tile([128, 512], F32R, name=f"wt{ft}", tag=f"wt{ft}") for ft in range(CT)]
                for ft in range(CT):
                    nc.sync.dma_start(
                        out=wt[ft],
                        in_=wqkvT[ft * 128:(ft + 1) * 128, ch * 512:(ch + 1) * 512])
                ntiles = NQT if is_q else NT
                for nt in range(ntiles):
                    ps = ps1.tile([128, 512], F32)
                    for ft in range(CT):
                        lhsT = (xTq_t[ft] if is_q else xT_t[ft])[:, nt * 128:(nt + 1) * 128]
                        nc.tensor.matmul(ps[:, :], lhsT, wt[ft][:, :],
                                         start=(ft == 0),
                                         stop=(ft == CT - 1 and not has_bqkv))
                    if has_bqkv:
                        nc.tensor.matmul(
                            ps[:, :], ones_row[:, :],
                            bqkv_sb[:, ch * 512:(ch + 1) * 512],
                            start=False, stop=True)
                    half = ch % 2
                    if is_q:
                        nc.scalar.copy(q_sb[nt][:, half * 512:(half + 1) * 512], ps[:, :])
                    elif is_k:
                        nc.scalar.copy(k_sb[nt][:, half * 512:(half + 1) * 512], ps[:, :])
                    else:
                        nc.vector.tensor_copy(
                            vput[nt][:, half * 8:half * 8 + 8, 0:D],
                            ps[:, :].rearrange("p (h d) -> p h d", h=8))

            # ---------------- S2: layernorm on q, k (free-dim per head) ----
            s1_stack.close()
            with tc.tile_pool(name="pln", bufs=3) as pln:
                for tiles, ntl, is_q_ln in ((q_sb, NQT, True), (k_sb, NT, False)):
                    for nt in range(ntl):
                        src = tiles[nt]
                        src3 = src[:, :].rearrange("p (h d) -> p h d", h=H)
                        sums = pln.tile([128, H], F32, name="sums", tag="sums")
                        nc.vector.reduce_sum(sums[:, :], src3, axis=mybir.AxisListType.X)
                        sq = pln.tile([128, C], F32, name="sq", tag="sq")
                        nc.scalar.square(sq[:, :], src[:, :])
                        sumsq = pln.tile([128, H], F32, name="sumsq", tag="sumsq")
                        nc.vector.reduce_sum(
                            sumsq[:, :], sq[:, :].rearrange("p (h d) -> p h d", h=H),
                            axis=mybir.AxisListType.X)
                        mean = pln.tile([128, H], F32, name="mean", tag="mean")
                        nc.vector.tensor_scalar_mul(mean[:, :], sums[:, :], 1.0 / D)
                        # var = sumsq/D - mean^2  (population variance)
                        msq = pln.tile([128, H], F32, name="msq", tag="msq")
                        nc.vector.tensor_mul(msq[:, :], mean[:, :], mean[:, :])
                        var = pln.tile([128, H], F32, name="var", tag="var")
                        nc.vector.scalar_tensor_tensor(
                            out=var[:, :], in0=sumsq[:, :], scalar=1.0 / D,
                            in1=msq[:, :], op0=OP.mult, op1=OP.subtract)
                        # rstd' = exp(-0.5*ln(var+eps) [+ ln(scale) for q])
                        lnv = pln.tile([128, H], F32, name="lnv", tag="lnv")
                        nc.scalar.activation(lnv[:, :], var[:, :], AF.Ln, bias=eps_t[:, :])
                        rstd = pln.tile([128, H], F32, name="rstd", tag="rstd")
                        nc.scalar.activation(rstd[:, :], lnv[:, :], AF.Exp,
                                             bias=(lnsc_t[:, :] if is_q_ln else 0.0),
                                             scale=-0.5)
                        dstR = src[:, :]
                        for h in range(H):
                            nc.vector.tensor_scalar(
                                out=dstR[:, h * D:(h + 1) * D],
                                in0=src[:, h * D:(h + 1) * D],
                                scalar1=mean[:, h:h + 1],
                                scalar2=rstd[:, h:h + 1],
                                op0=OP.subtract, op1=OP.mult)
                        if is_q_ln and has_qgb:
                            nc.vector.tensor_mul(dstR, dstR, qg_sb[:, :])
                            nc.vector.tensor_add(dstR, dstR, qb_sb[:, :])
                        if (not is_q_ln) and has_kgb:
                            nc.vector.tensor_mul(dstR, dstR, kg_sb[:, :])
                            nc.vector.tensor_add(dstR, dstR, kb_sb[:, :])

            # ---------------- S3: transpose q,k to head-major [d, n] -------
            patt = top.enter_context(tc.tile_pool(name="patt", bufs=1))
            qT = [patt.tile([128, NQ], F32R, name=f"qT{i}", tag=f"qT{i}") for i in range(CT)]
            kT = [patt.tile([128, N], F32R, name=f"kT{i}", tag=f"kT{i}") for i in range(CT)]
            with tc.tile_pool(name="ps3", bufs=2, space="PSUM") as ps3, \
                 tc.tile_pool(name="ps3k", bufs=2, space="PSUM") as ps3k:
                for ct in range(CT):
                    psq = ps3.tile([128, NQ], F32R)
                    for nt in range(NQT):
                        nc.tensor.transpose(
                            psq[:, nt * 128:(nt + 1) * 128],
                            q_sb[nt][:, ct * 128:(ct + 1) * 128],
                            identR[:, :])
                    nc.vector.tensor_copy(qT[ct][:, :], psq[:, :])
                    psk = ps3k.tile([128, N], F32R)
                    for nt in range(NT):
                        nc.tensor.transpose(
                            psk[:, nt * 128:(nt + 1) * 128],
                            k_sb[nt][:, ct * 128:(ct + 1) * 128],
                            identR[:, :])
                    nc.vector.tensor_copy(kT[ct][:, :], psk[:, :])

        # ---------------- S4: attnT = k q^T per head; exp; PV --------------
        # Barrier: S2's ACT Ln ops must not interleave with S4's Exp ops
        # (table-set ping-pong costs ~1.3us per reload), and S4's dense
        # matmul stream should start clean for HAM warm-up.
        tc.strict_bb_all_engine_barrier()
        # q_sb/k_sb raw values are dead after S3 - reuse their storage for the
        # (un)normalized attention outputs.
        aoutU = [q_sb[i // 2][:, (i % 2) * NQ:(i % 2) * NQ + NQ] for i in range(CT)]
        aoutN = [k_sb[i // 2][:, (i % 2) * NQ:(i % 2) * NQ + NQ] for i in range(CT)]
        sumsT = patt.tile([H, NQ], F32, name="sumsT", tag="sumsT")

        with tc.tile_pool(name="pexp", bufs=10) as pexp, \
             tc.tile_pool(name="pskq", bufs=3, space="PSUM") as pskq, \
             tc.tile_pool(name="pspv", bufs=2, space="PSUM") as pspv:
            for t in range(CT):  # head pair (2t, 2t+1)
                expTs = ([], [])
                for j in range(NT // 2):
                    pk = [pskq.tile([128, 1024], F32, name="pkq", tag="pkq")
                          for _ in range(2)]
                    for s in range(2):
                        mt = 2 * j + s
                        # adjacent matmuls on disjoint PE row groups (0-63 /
                        # 64-127) execute concurrently
                        for o in range(2):
                            nc.tensor.matmul(
                                pk[o][:, s * 512:(s + 1) * 512],
                                kT[t][o * 64:o * 64 + 64, mt * 128:(mt + 1) * 128],
                                qT[t][o * 64:o * 64 + 64, :],
                                start=True, stop=True)
                    for o in range(2):
                        et = pexp.tile([128, 1024], F32R, name="expT", tag="expT")
                        nc.scalar.activation(et[:, :], pk[o][:, :], AF.Exp)
                        expTs[o].append(et)
                for o in range(2):
                    h = 2 * t + o
                    ppv = pspv.tile([D + 1, NQ], F32, name="ppv", tag="ppv")
                    for mt in range(NT):
                        nc.tensor.matmul(
                            ppv[:, :],
                            vput[mt][:, h, :],
                            expTs[o][mt // 2][:, (mt % 2) * 512:(mt % 2) * 512 + 512],
                            start=(mt == 0), stop=(mt == NT - 1))
                    stmp = pexp.tile([1, NQ], F32, name="stmp", tag="stmp", bufs=3)
                    nc.vector.tensor_copy(stmp[:, :], ppv[D:D + 1, :])
                    nc.sync.dma_start(out=sumsT[h:h + 1, :], in_=stmp[:, :])
                    nc.vector.tensor_copy(aoutU[t][o * 64:o * 64 + 64, :], ppv[0:D, :])

        # softmax denominators -> recip (for PV normalize) + -logsum (for w)
        recipT = patt.tile([H, NQ], F32R, name="recipT", tag="recipT")
        nc.vector.reciprocal(recipT[:, :], sumsT[:, :])
        neglog = patt.tile([H, NQ], F32, name="neglog", tag="neglog")
        nc.scalar.activation(neglog[:, :], recipT[:, :], AF.Ln)
        nlT = [patt.tile([128, H], F32, name=f"nlT{i}", tag=f"nlT{i}") for i in range(NQT)]
        with tc.tile_pool(name="psnl", bufs=2, space="PSUM") as psnl, \
             tc.tile_pool(name="psbc", bufs=2, space="PSUM") as psbc:
            for nt in range(NQT):
                pnl = psnl.tile([128, H], F32)
                nc.tensor.transpose(
                    pnl[:, :], neglog[:, nt * 128:(nt + 1) * 128],
                    identR[:, :].bitcast(F32)[0:H, 0:H])
                nc.vector.tensor_copy(nlT[nt][:, :], pnl[:, :])
            for ct in range(CT):
                pbc = psbc.tile([128, NQ], F32)
                nc.tensor.matmul(pbc[:, :], ind_sb[:, ct, :], recipT[:, :],
                                 start=True, stop=True)
                nc.vector.tensor_mul(aoutN[ct], aoutU[ct], pbc[:, :])

        # ---------------- S5: attn = q k^T; w = Exp(attn - logsum); store --
        # ---------------- S6: out = attnout @ W_proj.T + b_proj ------------
        with tc.tile_pool(name="pw5", bufs=4) as pw5, \
             tc.tile_pool(name="pwp", bufs=1) as pwp, \
             tc.tile_pool(name="pob", bufs=2) as pob, \
             tc.tile_pool(name="psat", bufs=3, space="PSUM") as psat, \
             tc.tile_pool(name="pspj", bufs=2, space="PSUM") as pspj:
            for t in range(CT):  # head pair (2t, 2t+1)
                for nt in range(NQT):
                    pa = [psat.tile([128, 1024], F32, name="pat", tag="pat")
                          for _ in range(2)]
                    for s in range(2):
                        for o in range(2):
                            nc.tensor.matmul(
                                pa[o][:, s * 512:(s + 1) * 512],
                                qT[t][o * 64:o * 64 + 64, nt * 128:(nt + 1) * 128],
                                kT[t][o * 64:o * 64 + 64, s * 512:(s + 1) * 512],
                                start=True, stop=True)
                    for o in range(2):
                        h = 2 * t + o
                        wsb = pw5.tile([128, 1024], F32, name="wsb", tag="wsb")
                        nc.scalar.activation(wsb[:, :], pa[o][:, :], AF.Exp,
                                             bias=nlT[nt][:, h:h + 1])
                        nc.sync.dma_start(
                            out=wout[h, nt * 128:(nt + 1) * 128, :], in_=wsb[:, :])

            wp = [pwp.tile([128, C], F32R, name=f"wp{i}", tag=f"wp{i}") for i in range(CT)]
            for ct in range(CT):
                nc.sync.dma_start(out=wp[ct], in_=wprojT[ct * 128:(ct + 1) * 128, :])
            for nt in range(NQT):
                ob = pob.tile([128, C], F32, name="ob", tag="ob")
                for chalf in range(2):
                    pj = pspj.tile([128, 512], F32)
                    for ct in range(CT):
                        nc.tensor.matmul(
                            pj[:, :],
                            aoutN[ct][:, nt * 128:(nt + 1) * 128],
                            wp[ct][:, chalf * 512:(chalf + 1) * 512],
                            start=(ct == 0),
                            stop=(ct == CT - 1 and not has_bproj))
                    if has_bproj:
                        nc.tensor.matmul(pj[:, :], ones_row[:, :],
                                         bproj_sb[:, chalf * 512:(chalf + 1) * 512],
                                         start=False, stop=True)
                    nc.vector.tensor_copy(ob[:, chalf * 512:(chalf + 1) * 512], pj[:, :])
                nc.sync.dma_start(out=oout[nt * 128:(nt + 1) * 128, :], in_=ob[:, :])

    nc.compile()
    return nc


_NC_CACHE = {}


def _get_nc(flags):
    if flags not in _NC_CACHE:
        _NC_CACHE[flags] = _build(*flags)
    return _NC_CACHE[flags]


def _flags_of(inputs):
    return (bool(np.any(inputs["b_qkv"] != 0.0)),
            bool(np.any(inputs["b_proj"] != 0.0)),
            bool(np.any(inputs["qn_g"] != 1.0) or np.any(inputs["qn_b"] != 0.0)),
            bool(np.any(inputs["kn_g"] != 1.0) or np.any(inputs["kn_b"] != 0.0)))


def make_in_maps(inputs):
    inputs = {k: np.asarray(v, dtype=np.float32) for k, v in inputs.items()}
    x, W_qkv, b_qkv = inputs["x"], inputs["W_qkv"], inputs["b_qkv"]
    qn_g, qn_b = inputs["qn_g"], inputs["qn_b"]
    kn_g, kn_b = inputs["kn_g"], inputs["kn_b"]
    W_proj, b_proj = inputs["W_proj"], inputs["b_proj"]
    has_bqkv, has_bproj, has_qgb, has_kgb = _flags_of(inputs)

    wqkvT = np.ascontiguousarray(W_qkv.T)              # [C, 3C]
    wprojT = np.ascontiguousarray(W_proj.T)            # [C, C]
    ind = np.zeros((H, CT, 128), np.float32)
    for t in range(CT):
        ind[2 * t, t, 0:64] = 1.0
        ind[2 * t + 1, t, 64:128] = 1.0

    common = {"wqkvT": wqkvT, "wprojT": wprojT, "ind": ind,
              "idn": np.eye(128, dtype=np.float32),
              "onesr": np.ones((1, 128), np.float32),
              "vones": np.ones((128, H), np.float32)}
    if has_bqkv:
        common["bqkv"] = b_qkv.reshape(1, 3 * C)
    if has_bproj:
        common["bproj"] = b_proj.reshape(1, C)
    if has_qgb:
        scale = D ** -0.5
        common["qg"] = np.broadcast_to(np.tile(qn_g, H), (128, C)).copy()
        common["qb"] = np.broadcast_to(np.tile(qn_b * scale, H), (128, C)).copy()
    if has_kgb:
        common["kg"] = np.broadcast_to(np.tile(kn_g, H), (128, C)).copy()
        common["kb"] = np.broadcast_to(np.tile(kn_b, H), (128, C)).copy()

    in_maps = []
    for core in range(N_CORES):
        b, r = divmod(core, 2)
        xTb = np.ascontiguousarray(x[b].T)             # [C, N]
        xTqb = np.ascontiguousarray(xTb[:, r * NQ:(r + 1) * NQ])
        m = dict(common)
        m["xT"] = xTb
        m["xTq"] = xTqb
        in_maps.append(m)
    return in_maps


def kernel(x, W_qkv, b_qkv, qn_g, qn_b, kn_g, kn_b, W_proj, b_proj):
    inputs = dict(x=x, W_qkv=W_qkv, b_qkv=b_qkv, qn_g=qn_g, qn_b=qn_b,
                  kn_g=kn_g, kn_b=kn_b, W_proj=W_proj, b_proj=b_proj)
    inputs = {k: np.asarray(v, dtype=np.float32) for k, v in inputs.items()}
    nc = _get_nc(_flags_of(inputs))
    in_maps = make_in_maps(inputs)

    res = run_bass_kernel_spmd(nc, in_maps, core_ids=list(range(N_CORES)))

    out = np.empty((B, N, C), np.float32)
    weights = np.empty((B, H, N, N), np.float32)
    for core in range(N_CORES):
        b, r = divmod(core, 2)
        out[b, r * NQ:(r + 1) * NQ, :] = res.results[core]["out_part"]
        weights[b, :, r * NQ:(r + 1) * NQ, :] = res.results[core]["w_part"]
    return out, weights
